# revision 13
# baseline (speedup 1.0000x reference)
"""Trainium2 Bass kernel for nn_BitwiseLinear (8 NeuronCores, SPMD).

Reference semantics (B=32768, IN=OUT=1024):
    out = in_scale * weight_scale * (sign(x) @ sign(weight * gate_mask).T + bias)
    gate_mask = (sign(gate)+1)/2; in_scale = mean|x| per row; weight_scale = mean|w| per out.

Identities used:
    sign(weight * gate_mask) == sign(weight) * (gate >= 0)  (gate==0 -> mask 0.5 -> sign(w))
    out = sum|x|_row * (signmm + bias) * ws_eff,  ws_eff = sum|w|_row * 2^-20

Precision staging (rel-err budget 2e-2; measured ~2.7e-3 end to end):
    x/weight/gate staged to HBM as bf16 (signs and gate>=0 exact, mean|.|
    errors ~1e-4), out as bf16 (0.2%).  Per-core HBM traffic 36 -> 18 MB.

Sharding: data-parallel on batch across the 8 cores (x/out sharded 4096 rows
per core), weight/gate/bias replicated, no collectives.

Per core:
  prep: weight (+gate) bf16 -> Sign (ACT) [* (gate>=0) mask (DVE)] -> bf16
        w_bin; transposed to [i, o] by the DMA XBAR (16x128 hardware tile
        transpose); cast bf16 -> fp8 into the DoubleRow pair layout; row |w|
        sums -> ws_eff broadcast tile (bf16).
  loop over 16 groups of 2 x-tiles; transposes alternate between the DMA
  XBAR (even groups; DMA engines) and PE identity-matmul transposes (odd
  groups) to balance the two oversubscribed resources:
        group DMA (bf16, SWDGE queue) -> transpose -> ACT Sign -> fp8
        sign(x).T; per tile 4 fp8 DoubleRow matmuls (K=256) x 2 PSUM banks;
        row |x| sums via one grouped DVE tensor_reduce or per-tile ACT
        Abs+accum_out (alternating groups); one fused custom-DVE op
        (TENSOR_TENSOR_REDUCE: psum * ws_bcast * is_raw -> bf16) drains each
        tile straight into the output group buffer.
  Runtime specialization: the bias matmuls are dropped when bias is all-zero
  and the gate path is dropped when gate >= 0 everywhere (both checked against
  the actual inputs; other variants compile lazily and remain correct).
"""

import numpy as np
import ml_dtypes

import concourse.bacc as bacc
import concourse.mybir as mybir
import concourse.tile as tile
from concourse import masks
from concourse.bass_utils import run_bass_kernel_spmd
from concourse.dve_ops import TENSOR_TENSOR_REDUCE

B, IN, OUT = 32768, 1024, 1024
NCORES = 8
BSH = B // NCORES            # 4096 rows per core
P = 128                      # partitions
NT = BSH // P                # 32 x-tiles per core
KC = IN // P                 # 8 contraction chunks of 128
NPAIR = KC // 2              # 4 DoubleRow K-pairs (256 each)
NCH = 512                    # one PSUM bank of f32
F32 = mybir.dt.float32
BF16 = mybir.dt.bfloat16
FP8 = mybir.dt.float8e4
WS_SCALE = float(2.0 ** -20)  # 1/(1024*1024): folds both mean divisors

_CACHE: dict = {}


def _build(with_bias=True, with_gate=True):
    nc = bacc.Bacc("TRN2", target_bir_lowering=False, debug=False,
                   num_devices=NCORES)

    x_ext = nc.declare_dram_parameter("x", [BSH, IN], BF16, isOutput=False)
    w_ext = nc.declare_dram_parameter("weight", [OUT, IN], BF16, isOutput=False)
    g_ext = nc.declare_dram_parameter("gate", [OUT, IN], BF16, isOutput=False)
    b_ext = nc.declare_dram_parameter("bias", [1, OUT], F32, isOutput=False)
    o_ext = nc.declare_dram_parameter("out", [BSH, OUT], BF16, isOutput=True)

    x_ap = x_ext.ap()
    w_ap = w_ext.ap()
    g_ap = g_ext.ap()
    b_ap = b_ext.ap()
    o_ap = o_ext.ap()

    ACT = mybir.ActivationFunctionType
    ALU = mybir.AluOpType
    AX = mybir.AxisListType
    DR = mybir.MatmulPerfMode.DoubleRow

    with tile.TileContext(nc) as tc:
        with tc.tile_pool(name="const", bufs=1) as cp:
            ident_f32 = cp.tile([P, P], F32)
            ident_bf = cp.tile([P, P], BF16)
            ones_f8 = cp.tile([1, P], FP8)
            ones_f32 = cp.tile([1, P], F32)
            zbias = cp.tile([P, 1], F32)

            # persistent prepped weights
            # pair j holds binarized wT chunks 2j (at [:, :OUT]) and 2j+1
            wtq = [cp.tile([P, 2 * OUT], FP8, tag=f"wtq{j}", name=f"wtq{j}") for j in range(NPAIR)]
            bias_f8 = cp.tile([1, OUT], FP8)      # raw bias (fp8) added pre-scale
            ws_bcast = cp.tile([P, OUT], BF16)    # ws * 2^-20 broadcast over partitions

            # ---------------- weight prep (replicated on every core) --------
            with tc.tile_pool(name="wprep", bufs=2) as wp, \
                 tc.tile_pool(name="wkeep", bufs=1) as wk, \
                 tc.tile_pool(name="xin", bufs=5) as xin_pool, \
                 tc.tile_pool(name="xtr", bufs=2) as xtr_pool, \
                 tc.tile_pool(name="xbt", bufs=3) as xbt_pool, \
                 tc.tile_pool(name="xb1", bufs=6) as xb1_pool, \
                 tc.tile_pool(name="opair", bufs=3) as opair_pool, \
                 tc.tile_pool(name="sc", bufs=10) as sc_pool, \
                 tc.tile_pool(name="scr", bufs=3) as scr_pool, \
                 tc.tile_pool(name="pst", bufs=2, space="PSUM") as pst_pool, \
                 tc.tile_pool(name="pso", bufs=3, space="PSUM") as pso_pool:
                # group granularity: G tiles (128 rows each) per DMA / reduce
                G = 2
                NG = NT // G
                xtgs = [None] * NG       # raw [b, i] bf16 group tiles
                xbts = [None] * NT       # per-tile fp8 sign(x).T APs
                is_gs = [None] * NG
                is_raws = [None] * NT
                out_groups = [None] * NG

                pending = {}

                def stage_front_group(m, sign_now=True):
                    """Group DMA (SWDGE); transpose via XBAR (even m) or PE
                    (odd m); Signs optionally deferred (prologue: weight Signs
                    go first on the in-order ACT queue)."""
                    xtg = xin_pool.tile([P, G * IN], BF16, tag="xtg",
                                        name=f"xtg{m}")
                    nc.sync.dma_start(
                        xtg[:].rearrange("p (t i) -> p t i", t=G),
                        x_ap[m * G * P:(m + 1) * G * P, :].rearrange(
                            "(t p) i -> p t i", p=P))
                    xtgs[m] = xtg
                    if sign_now:
                        prep_group(m)

                def prep_group(m):
                    """Transpose + Sign for group m.  Emitted at least one
                    full group after the group's DMA was issued so the
                    in-order queues never stall on the DMA semaphore."""
                    xtg = xtgs[m]
                    if m % 2 == 0:
                        # XBAR: xtr[p, c, b] = xtg[b, c*128+p], c = G*8 chunks
                        # (issued from the ACT queue: hwdge engines are SP and
                        # Activation only, and SP carries the x-in stream)
                        xtr = xtr_pool.tile([P, G * IN], BF16, tag="xtr",
                                            name=f"xtr{m}")
                        nc.sync.dma_start_transpose(
                            xtr[:].rearrange("p (c b) -> p c b", b=P), xtg[:])
                        xb = xbt_pool.tile([P, G * IN], FP8, tag="xbT",
                                           name=f"xbT{m}")
                        nc.scalar.activation(xb[:], xtr[:], ACT.Sign,
                                             bias=zbias[:])
                        for t in range(G):
                            xbts[m * G + t] = xb[:, t * IN:(t + 1) * IN]
                    else:
                        for t in range(G):
                            ps_t = pst_pool.tile([P, IN], BF16, tag="ps_t",
                                                 name=f"ps_t{m}_{t}")
                            xt = xtg[:, t * IN:(t + 1) * IN]
                            for c in range(KC):
                                nc.tensor.transpose(ps_t[:, c * P:(c + 1) * P],
                                                    xt[:, c * P:(c + 1) * P],
                                                    ident_bf[:])
                            xb = xb1_pool.tile([P, IN], FP8, tag="xb1",
                                               name=f"xb1_{m}_{t}")
                            nc.scalar.activation(xb[:], ps_t[:], ACT.Sign,
                                                 bias=zbias[:])
                            xbts[m * G + t] = xb[:]

                def emit_reduce_group(m):
                    """|x| row sums for group m: even groups one DVE grouped
                    tensor_reduce, odd groups per-tile ACT Abs+accum_out."""
                    if m >= NG or is_gs[m] is not None:
                        return
                    xtg = xtgs[m]
                    if m % 4 != 1:
                        is_g = sc_pool.tile([P, G], F32, tag="is_g",
                                            name=f"is_g{m}")
                        nc.vector.tensor_reduce(
                            is_g[:], xtg[:].rearrange("p (t i) -> p t i", t=G),
                            axis=AX.X, op=ALU.add, apply_absolute_value=True)
                        is_gs[m] = is_g
                        for t in range(G):
                            is_raws[m * G + t] = is_g[:, t:t + 1]
                    else:
                        is_gs[m] = True
                        for t in range(G):
                            it = m * G + t
                            is_raw = sc_pool.tile([P, 1], F32, tag="is_raw",
                                                  name=f"is_raw{it}")
                            scr = scr_pool.tile([P, IN], BF16, tag="abs_scr",
                                                name="abs_scr")
                            nc.scalar.activation(scr[:],
                                                 xtg[:, t * IN:(t + 1) * IN],
                                                 ACT.Abs, bias=zbias[:],
                                                 accum_out=is_raw[:])
                            is_raws[it] = is_raw[:]

                masks.make_identity(nc, ident_f32[:])
                masks.make_identity(nc, ident_bf[:])
                nc.gpsimd.memset(ones_f8[:], 1.0)
                nc.gpsimd.memset(ones_f32[:], 1.0)
                nc.gpsimd.memset(zbias[:], 0.0)

                wt4 = [wk.tile([P, 4 * IN], BF16, tag=f"wt4_{q}",
                               name=f"wt4_{q}") for q in range(2)]
                gt4 = []
                for t in range(KC):
                    nc.gpsimd.dma_start(
                        wt4[t // 4][:, (t % 4) * IN:((t % 4) + 1) * IN],
                        w_ap[t * P:(t + 1) * P, :])
                for q in range(2):
                    if with_gate:
                        g4 = wk.tile([P, 4 * IN], BF16, tag=f"gt4_{q}",
                                     name=f"gt4_{q}")
                        nc.gpsimd.dma_start(
                            g4[:].rearrange("p (t i) -> p t i", t=4),
                            g_ap[q * 4 * P:(q + 1) * 4 * P, :].rearrange(
                                "(t p) i -> p t i", p=P))
                        gt4.append(g4)

                stage_front_group(0, sign_now=False)
                stage_front_group(1, sign_now=False)
                stage_front_group(2, sign_now=False)
                # ---- weight prep: weight Signs are emitted BEFORE the
                # prologue x Signs so the wtq chain (which gates the first
                # matmuls) owns the head of the ACT queue ----
                # o-tile t: [128 o_t, 1024 i]
                w_bin = [wk.tile([P, IN], BF16, tag=f"wbin{t}", name=f"wbin{t}")
                         for t in range(KC)]
                ws_cols = wk.tile([P, KC], F32)   # per-o |w| row sums, tile t in col t
                bias_sb = wk.tile([1, OUT], F32)
                ws_row = wk.tile([1, OUT], F32)
                if with_bias:
                    nc.sync.dma_start(bias_sb[:], b_ap[:, :])
                    nc.vector.tensor_copy(bias_f8[:], bias_sb[:])

                for t in range(KC):
                    wt = wt4[t // 4][:, (t % 4) * IN:((t % 4) + 1) * IN]
                    nc.vector.tensor_reduce(ws_cols[:, t:t + 1], wt, axis=AX.X,
                                            op=ALU.add, apply_absolute_value=True)
                    if with_gate:
                        gt = gt4[t // 4][:, (t % 4) * IN:((t % 4) + 1) * IN]
                        sgn = wp.tile([P, IN], BF16)
                        nc.scalar.activation(sgn[:], wt, ACT.Sign, bias=zbias[:])
                        msk = wp.tile([P, IN], BF16)
                        nc.vector.tensor_scalar(msk[:], gt, 0.0, None,
                                                op0=ALU.is_ge)
                        nc.vector.tensor_tensor(w_bin[t][:], sgn[:], msk[:],
                                                op=ALU.mult)
                    else:
                        nc.scalar.activation(w_bin[t][:], wt, ACT.Sign,
                                             bias=zbias[:])

                # x transposes+Signs for the prologue groups go after the
                # weight Signs
                prep_group(0)
                prep_group(1)

                # wT via XBAR: wtr[q][p, (s c b)] = w_bin[4q+s][b, c*128+p]
                wtrs = [wk.tile([P, 4 * IN], BF16, tag=f"wtr{q}",
                                name=f"wtr{q}") for q in range(2)]
                for ot in range(KC):
                    q, s = divmod(ot, 4)
                    nc.sync.dma_start_transpose(
                        wtrs[q][:, s * IN:(s + 1) * IN].rearrange(
                            "p (c b) -> p c b", b=P),
                        w_bin[ot][:])
                # cast bf16 -> fp8 into the DR pair layout:
                # wtq[c//2][p, (c%2)*OUT + ot*128 + b] = swT[c*128+p, ot*128+b]
                for c in range(KC):
                    for q in range(2):
                        src = wtrs[q][:].rearrange(
                            "p (s c b) -> p s c b", s=4, c=KC)[:, :, c, :]
                        dst = wtq[c // 2][:].rearrange(
                            "p (h o) -> p h o", h=2)[:, c % 2, :].rearrange(
                            "p (s b) -> p s b", s=KC)[:, 4 * q:4 * q + 4, :]
                        if c < 2:
                            nc.vector.tensor_copy(dst, src)
                        else:
                            nc.scalar.copy(dst, src)

                # ws_row[0, o] = sum_i |w[o, i]| * 2^-20, via 8 tiny PE transposes
                for half in range(2):
                    ps_row = pso_pool.tile([1, NCH], F32, tag="ps_o",
                                           name=f"ps_row{half}")
                    for tt in range(4):
                        t = half * 4 + tt
                        nc.tensor.transpose(ps_row[0:1, tt * P:(tt + 1) * P],
                                            ws_cols[:, t:t + 1], ident_f32[:])
                    nc.scalar.activation(ws_row[:, half * NCH:(half + 1) * NCH],
                                         ps_row[:], ACT.Copy, scale=WS_SCALE)

                # broadcast ws_row across partitions with a K=1 matmul
                for n in range(OUT // NCH):
                    ps_bc = pso_pool.tile([P, NCH], F32, tag="ps_o",
                                          name=f"ps_bc{n}")
                    nc.tensor.matmul(ps_bc[:], ones_f32[:],
                                     ws_row[:, n * NCH:(n + 1) * NCH])
                    nc.vector.tensor_copy(ws_bcast[:, n * NCH:(n + 1) * NCH],
                                          ps_bc[:])

                for it in range(NT):
                    m, t = divmod(it, G)
                    if it == 0:
                        emit_reduce_group(0)
                        emit_reduce_group(1)
                    xb = xbts[it]
                    if t == 0:
                        out_groups[m] = opair_pool.tile([P, G * OUT], BF16,
                                                        tag="og", name=f"og{m}")
                    ps_o = pso_pool.tile([P, OUT], F32, tag="ps_o",
                                         name=f"ps_o{it}")
                    for j in range(NPAIR):
                        xp = xb[:, 2 * j * P:(2 * j + 2) * P].rearrange(
                            "p (two m) -> p two m", two=2)
                        wq = wtq[j][:].rearrange("p (two o) -> p two o", two=2)
                        for n in range(OUT // NCH):
                            nc.tensor.matmul(
                                ps_o[:, n * NCH:(n + 1) * NCH],
                                xp,
                                wq[:, :, n * NCH:(n + 1) * NCH],
                                start=(j == 0),
                                stop=(not with_bias and j == NPAIR - 1),
                                perf_mode=DR)
                    if with_bias:
                        for n in range(OUT // NCH):
                            nc.tensor.matmul(ps_o[:, n * NCH:(n + 1) * NCH],
                                             ones_f8[:],
                                             bias_f8[:, n * NCH:(n + 1) * NCH],
                                             start=False, stop=True)
                    # fused drain: out = psum * ws_bcast * is_raw -> bf16,
                    # straight into the output group buffer (one DVE op)
                    nc.vector._custom_dve(
                        TENSOR_TENSOR_REDUCE,
                        out=out_groups[m][:, t * OUT:(t + 1) * OUT],
                        in0=ps_o[:],
                        in1=ws_bcast[:],
                        s0=0.0,
                        s1=is_raws[it])
                    if t == G - 1:
                        # output DMAs on the SWDGE queue; the last few groups
                        # on the (by now idle) sync ring
                        eng = nc.sync if m >= NG - 3 else nc.gpsimd
                        eng.dma_start(
                            o_ap[m * G * P:(m + 1) * G * P, :].rearrange(
                                "(u p) o -> p u o", p=P),
                            out_groups[m][:].rearrange(
                                "p (u o) -> p u o", u=G))
                    if t == 0:
                        if m + 3 < NG:
                            stage_front_group(m + 3, sign_now=False)
                        if m + 2 < NG:
                            prep_group(m + 2)
                        emit_reduce_group(m + 2)

    nc.compile()
    return nc


def _get_nc(with_bias, with_gate):
    key = f"nc{int(with_bias)}{int(with_gate)}"
    if key not in _CACHE:
        _CACHE[key] = _build(with_bias, with_gate)
    return _CACHE[key]


def run(x, weight, gate, bias, trace=False):
    # gate >= 0 everywhere makes the gate mask exactly 1 ((sign(g)+1)/2 with
    # g==0 -> 0.5, and sign(w*0.5) == sign(w)); skip it entirely then.
    gate_np = np.asarray(gate)
    nc = _get_nc(bool(np.any(np.asarray(bias))),
                 not bool(np.all(gate_np >= 0.0)))
    x8 = np.ascontiguousarray(
        np.asarray(x, dtype=np.float32)).astype(ml_dtypes.bfloat16)
    wbf = np.ascontiguousarray(
        np.asarray(weight, dtype=np.float32)).astype(ml_dtypes.bfloat16)
    gbf = np.ascontiguousarray(
        gate_np.astype(np.float32)).astype(ml_dtypes.bfloat16)
    bias = np.ascontiguousarray(np.asarray(bias, dtype=np.float32)).reshape(1, OUT)
    in_maps = [
        {"x": x8[i * BSH:(i + 1) * BSH], "weight": wbf, "gate": gbf, "bias": bias}
        for i in range(NCORES)
    ]
    res = run_bass_kernel_spmd(nc, in_maps, core_ids=list(range(NCORES)), trace=trace)
    out = np.concatenate([res.results[i]["out"] for i in range(NCORES)], axis=0)
    return np.asarray(out).astype(np.float32), res


def kernel(x, weight, gate, bias):
    out, _ = run(x, weight, gate, bias, trace=False)
    return out
